# revision 54
# baseline (speedup 1.0000x reference)
"""Block-diagonal linear for TRN2, 8 NeuronCores.

y = concat_h(x_h @ w_h + b_h) with x:[4,4096,4096] split into 16 blocks of
256 features; w:[16,256,256]; b:[16,256].

Sharding: data-parallel over rows. x is reshaped to [16384, 4096] and each
core takes 2048 contiguous rows; w and b are replicated. Zero communication.

Precision: x and w are cast to bf16 on the host; the PE accumulates in
fp32 PSUM and y leaves the chip as bf16 (host upcasts). Per-core HBM
traffic drops from 67 MB (fp32 in/out) to ~36 MB, which matters because
all DMA shares a ~360 GB/s per-core ceiling (16 engines x ~22.5 GB/s),
and every PE op streams at the full 1 column/cycle bf16 rate (fp32
streams at 1/4 rate). Measured rel error vs the fp32 reference is
3.7e-3 of absmax, well within the 2e-2 gate.

Per-core kernel (Tile framework):
  - w is staged in SBUF as [128, 16, 2, 256] (contraction dim on partitions)
    via the Pool (gpsimd) software DGE, whose descriptor generation runs on
    the otherwise-idle Pool sequencer; b is staged + partition-broadcast
    first so the DVE bias-adds aren't gated at startup.
  - For each 128-row tile of x: one contiguous 1 MB DMA on the SP ring
    (partition p = row p, 8 KB sequential per partition — optimal HBM
    locality), then per group of 4 feature-chunks: PE-transpose them
    (features onto partitions), copy PSUM->SBUF on ACT, and run the 2-step
    accumulated bf16 matmuls for the 2 feature blocks they feed; DVE adds
    the bias while evicting PSUM->SBUF (cast to bf16); y goes out in two
    half-tile DMAs per tile on the Pool (gpsimd) software-DGE ring, whose
    sequencer is idle in steady state so y issues never contend with the
    ACT evictions.
"""

import numpy as np

import concourse.bacc as bacc
import concourse.mybir as mybir
from concourse import bass2jax, tile
from concourse.masks import make_identity

N_CORES = 8
ROWS_TOTAL = 4 * 4096
ROWS = ROWS_TOTAL // N_CORES  # 2048 rows per core
WIDTH = 4096
NB = 16  # feature blocks
BW = 256  # block width
P = 128
M_TILES = ROWS // P  # 16

FP32 = mybir.dt.float32
BF16 = mybir.dt.bfloat16


def _build(repeat=1, xb=4, xtb=5, yb=5, ptb=2, pyb=6):
    nc = bacc.Bacc(None, target_bir_lowering=False, debug=False)
    x = nc.dram_tensor("x", [ROWS, WIDTH], BF16, kind="ExternalInput")
    # w arrives host-pre-transposed as [p, h, ic, j] = w[h, ic*128+p, j]
    # flattened to [128, 8192]: its staging DMA is 16 KB/partition contiguous
    # (128 descriptors) instead of ~16k scattered 512 B descriptors.
    w = nc.dram_tensor("w", [P, NB * 2 * BW], BF16, kind="ExternalInput")
    b = nc.dram_tensor("b", [NB, BW], FP32, kind="ExternalInput")
    y = nc.dram_tensor("y", [ROWS, WIDTH], BF16, kind="ExternalOutput")

    with tile.TileContext(nc) as tc:
        with (
            tc.tile_pool(name="const", bufs=1) as const_pool,
            tc.tile_pool(name="xpool", bufs=xb) as x_pool,
            tc.tile_pool(name="xtpool", bufs=xtb) as xt_pool,
            tc.tile_pool(name="ypool", bufs=yb) as y_pool,
            tc.tile_pool(name="pt", bufs=ptb, space="PSUM") as psum_t,
            tc.tile_pool(name="py", bufs=pyb, space="PSUM") as psum_y,
        ):
            ident_f = const_pool.tile([P, P], FP32)
            make_identity(nc, ident_f[:])
            # PE consumes the identity in bf16 (0.0/1.0 are exact).
            ident = const_pool.tile([P, P], BF16)
            nc.scalar.copy(ident[:], ident_f[:])

            # Bias first on the gpsimd queue: the DVE bias-adds need b_rep
            # early, and the broadcast is quick; the w staging follows.
            # b_rep[p, h, j] = b[h, j].
            b_lin = const_pool.tile([1, NB, BW], FP32)
            nc.gpsimd.dma_start(
                b_lin[:], b.rearrange("(one h) j -> one h j", one=1)
            )
            b_rep = const_pool.tile([P, NB, BW], FP32)
            nc.gpsimd.partition_broadcast(
                b_rep[:].rearrange("p h j -> p (h j)"),
                b_lin[:].rearrange("o h j -> o (h j)"),
            )

            # w_sb[p, h, ic, j] = w[h, ic*128+p, j]; contiguous per partition
            # (host pre-transposed). Two chunks so the first matmuls only
            # wait for blocks 0-7.
            w_sb = const_pool.tile([P, NB, 2, BW], BF16)
            w_re = w.rearrange("p (h ic j) -> p h ic j", h=NB, ic=2)
            for q in range(2):
                nc.scalar.dma_start(
                    w_sb[:, 8 * q:8 * q + 8], w_re[:, 8 * q:8 * q + 8]
                )

            x_rows = x.rearrange("(t p) i -> t p i", p=P)
            y_rows = y.rearrange("(t p) i -> t p i", p=P)

            import contextlib

            rep_ctx = (
                tc.For_i(0, repeat, 1) if repeat > 1 else contextlib.nullcontext()
            )
            with rep_ctx:
                _main_loop(nc, tc, locals())

    nc.compile()
    return nc


def _main_loop(nc, tc, env):
    x_pool = env["x_pool"]
    xt_pool = env["xt_pool"]
    y_pool = env["y_pool"]
    psum_t = env["psum_t"]
    psum_y = env["psum_y"]
    ident = env["ident"]
    w_sb = env["w_sb"]
    b_rep = env["b_rep"]
    x_rows = env["x_rows"]
    y_rows = env["y_rows"]
    if True:
            for mi in range(M_TILES):
                # One contiguous 1 MB read: partition p <- row p (8 KB each).
                # Tile 0 loads in halves so the PE can start sooner.
                x_t = x_pool.tile([P, WIDTH], BF16)
                if mi == 0:
                    hw = WIDTH // 2
                    for q in range(2):
                        nc.sync.dma_start(
                            x_t[:, q * hw:(q + 1) * hw],
                            x_rows[mi][:, q * hw:(q + 1) * hw],
                        )
                else:
                    nc.sync.dma_start(x_t[:], x_rows[mi])

                # Per group g: transpose chunks 4g..4g+3 (features onto
                # partitions), then immediately the matmuls for blocks
                # 2g, 2g+1 which consume exactly those chunks.
                xT = xt_pool.tile([P, WIDTH // P, P], BF16)
                y_t = y_pool.tile([P, NB, BW], BF16)
                for g in range(8):
                    pt = psum_t.tile([P, 4, P], BF16, tag="pt")
                    for k in range(4):
                        c = 4 * g + k
                        nc.tensor.transpose(
                            pt[:, k, :], x_t[:, c * P:(c + 1) * P], ident[:]
                        )
                    nc.scalar.copy(xT[:, 4 * g:4 * g + 4, :], pt[:])

                    py = psum_y.tile([P, 2, BW], FP32)
                    for u in range(2):
                        h = 2 * g + u
                        nc.tensor.matmul(
                            py[:, u, :], xT[:, 2 * h, :], w_sb[:, h, 0, :],
                            start=True, stop=False,
                        )
                        nc.tensor.matmul(
                            py[:, u, :], xT[:, 2 * h + 1, :], w_sb[:, h, 1, :],
                            start=False, stop=True,
                        )
                    nc.vector.tensor_add(
                        y_t[:, 2 * g:2 * g + 2, :],
                        py[:],
                        b_rep[:, 2 * g:2 * g + 2, :],
                    )
                    if mi == M_TILES - 1:
                        # Last tile: stream each group's slice out right after
                        # its bias-add so the kernel tail is just one small
                        # DMA instead of a full row. Pool (gpsimd) ring: its
                        # sequencer is idle in steady state, so y issues never
                        # contend with the ACT evictions.
                        nc.gpsimd.dma_start(
                            y_rows[mi][:, g * 512:(g + 1) * 512],
                            y_t[:].rearrange("p h j -> p (h j)")[
                                :, g * 512:(g + 1) * 512
                            ],
                        )
                if mi != M_TILES - 1:
                    y_flat = y_t[:].rearrange("p h j -> p (h j)")
                    hw = WIDTH // 2
                    for q in range(2):
                        nc.gpsimd.dma_start(
                            y_rows[mi][:, q * hw:(q + 1) * hw],
                            y_flat[:, q * hw:(q + 1) * hw],
                        )


class _Runner:
    """Compile once, keep the jitted SPMD executable for reuse."""

    def __init__(self, repeat=1):
        import jax
        from jax.experimental.shard_map import shard_map
        from jax.sharding import Mesh, PartitionSpec

        self.jax = jax
        nc = _build(repeat=repeat)
        bass2jax.install_neuronx_cc_hook()

        assert nc.dbg_addr is None
        part_name = (
            nc.partition_id_tensor.name if nc.partition_id_tensor else None
        )
        in_names, out_names, out_avals = [], [], []
        for alloc in nc.m.functions[0].allocations:
            if not isinstance(alloc, mybir.MemoryLocationSet):
                continue
            name = alloc.memorylocations[0].name
            if alloc.kind == "ExternalInput":
                if name != part_name:
                    in_names.append(name)
            elif alloc.kind == "ExternalOutput":
                out_names.append(name)
                out_avals.append(
                    jax.core.ShapedArray(
                        tuple(alloc.tensor_shape), mybir.dt.np(alloc.dtype)
                    )
                )
        self.in_names = list(in_names)
        self.out_names = out_names
        self.out_avals = out_avals
        n_params = len(in_names)
        n_outs = len(out_names)
        all_names = in_names + out_names
        if part_name is not None:
            all_names = all_names + [part_name]

        def _body(*args):
            operands = list(args)
            if part_name is not None:
                operands.append(bass2jax.partition_id_tensor())
            outs = bass2jax._bass_exec_p.bind(
                *operands,
                out_avals=tuple(out_avals),
                in_names=tuple(all_names),
                out_names=tuple(out_names),
                lowering_input_output_aliases=(),
                sim_require_finite=True,
                sim_require_nnan=True,
                nc=nc,
            )
            return tuple(outs)

        devices = jax.devices()[:N_CORES]
        assert len(devices) == N_CORES
        self.mesh = Mesh(np.asarray(devices), ("core",))
        in_specs = (PartitionSpec("core"),) * (n_params + n_outs)
        out_specs = (PartitionSpec("core"),) * n_outs
        self.donate = tuple(range(n_params, n_params + n_outs))
        self.fn = jax.jit(
            shard_map(
                _body,
                mesh=self.mesh,
                in_specs=in_specs,
                out_specs=out_specs,
                check_rep=False,
            ),
            donate_argnums=self.donate,
            keep_unused=True,
        )

    def zeros(self):
        return [
            np.zeros((N_CORES * a.shape[0], *a.shape[1:]), a.dtype)
            for a in self.out_avals
        ]

    def prep(self, x, w, b):
        """Global (concatenated-over-cores) input arrays, in in_names order."""
        import jax
        import jax.numpy as jnp

        with jax.default_device(jax.devices("cpu")[0]):
            x2 = np.asarray(
                jnp.asarray(np.asarray(x).reshape(ROWS_TOTAL, WIDTH)).astype(
                    jnp.bfloat16
                )
            )
            wb = np.asarray(jnp.asarray(np.asarray(w)).astype(jnp.bfloat16))
        # w[h, ic*128+p, j] -> [p, (h, ic, j)]: per-partition-contiguous.
        wt = np.ascontiguousarray(
            wb.reshape(NB, 2, P, BW).transpose(2, 0, 1, 3).reshape(P, NB * 2 * BW)
        )
        b = np.ascontiguousarray(np.asarray(b, dtype=np.float32))
        per = {
            "x": x2,
            "w": np.concatenate([wt] * N_CORES, axis=0),
            "b": np.concatenate([b] * N_CORES, axis=0),
        }
        return [per[n] for n in self.in_names]

    def __call__(self, ins, zeros):
        outs = self.fn(*ins, *zeros)
        return dict(zip(self.out_names, outs))


_RUNNER = None


def _get_runner():
    global _RUNNER
    if _RUNNER is None:
        _RUNNER = _Runner()
    return _RUNNER


def kernel(x, w, b):
    r = _get_runner()
    outs = r(r.prep(x, w, b), r.zeros())
    y = np.asarray(outs["y"]).astype(np.float32)
    return y.reshape(4, 4096, WIDTH)


# revision 55
# speedup vs baseline: 1.1310x; 1.1310x over previous
"""Block-diagonal linear for TRN2, 8 NeuronCores.

y = concat_h(x_h @ w_h + b_h) with x:[4,4096,4096] split into 16 blocks of
256 features; w:[16,256,256]; b:[16,256].

Sharding: data-parallel over rows. x is reshaped to [16384, 4096] and each
core takes 2048 contiguous rows; w and b are replicated. Zero communication.

Precision: x and w are cast to bf16 on the host; the PE accumulates in
fp32 PSUM and y leaves the chip as bf16 (host upcasts). Per-core HBM
traffic drops from 67 MB (fp32 in/out) to ~36 MB, which matters because
all DMA shares a ~360 GB/s per-core ceiling (16 engines x ~22.5 GB/s),
and every PE op streams at the full 1 column/cycle bf16 rate (fp32
streams at 1/4 rate). Measured rel error vs the fp32 reference is
3.7e-3 of absmax, well within the 2e-2 gate.

Per-core kernel (Tile framework):
  - w is staged in SBUF as [128, 16, 2, 256] (contraction dim on partitions)
    via the Pool (gpsimd) software DGE, whose descriptor generation runs on
    the otherwise-idle Pool sequencer; b is staged + partition-broadcast
    first so the DVE bias-adds aren't gated at startup.
  - For each 128-row tile of x: one contiguous 1 MB DMA on the SP ring
    (partition p = row p, 8 KB sequential per partition — optimal HBM
    locality), then per group of 4 feature-chunks: PE-transpose them
    (features onto partitions), copy PSUM->SBUF on ACT, and run the 2-step
    accumulated bf16 matmuls for the 2 feature blocks they feed; DVE adds
    the bias while evicting PSUM->SBUF (cast to bf16); y goes out in two
    half-tile DMAs per tile on the Pool (gpsimd) software-DGE ring, whose
    sequencer is idle in steady state so y issues never contend with the
    ACT evictions.
"""

import numpy as np

import concourse.bacc as bacc
import concourse.mybir as mybir
from concourse import bass2jax, tile
from concourse.masks import make_identity

N_CORES = 8
ROWS_TOTAL = 4 * 4096
ROWS = ROWS_TOTAL // N_CORES  # 2048 rows per core
WIDTH = 4096
NB = 16  # feature blocks
BW = 256  # block width
P = 128
M_TILES = ROWS // P  # 16

FP32 = mybir.dt.float32
BF16 = mybir.dt.bfloat16


def _build(repeat=1, xb=4, xtb=5, yb=5, ptb=3, pyb=5):
    nc = bacc.Bacc(None, target_bir_lowering=False, debug=False)
    x = nc.dram_tensor("x", [ROWS, WIDTH], BF16, kind="ExternalInput")
    # w arrives host-pre-transposed as [p, h, ic, j] = w[h, ic*128+p, j]
    # flattened to [128, 8192]: its staging DMA is 16 KB/partition contiguous
    # (128 descriptors) instead of ~16k scattered 512 B descriptors.
    w = nc.dram_tensor("w", [P, NB * 2 * BW], BF16, kind="ExternalInput")
    b = nc.dram_tensor("b", [NB, BW], FP32, kind="ExternalInput")
    y = nc.dram_tensor("y", [ROWS, WIDTH], BF16, kind="ExternalOutput")

    with tile.TileContext(nc) as tc:
        with (
            tc.tile_pool(name="const", bufs=1) as const_pool,
            tc.tile_pool(name="xpool", bufs=xb) as x_pool,
            tc.tile_pool(name="xtpool", bufs=xtb) as xt_pool,
            tc.tile_pool(name="ypool", bufs=yb) as y_pool,
            tc.tile_pool(name="pt", bufs=ptb, space="PSUM") as psum_t,
            tc.tile_pool(name="py", bufs=pyb, space="PSUM") as psum_y,
        ):
            ident_f = const_pool.tile([P, P], FP32)
            make_identity(nc, ident_f[:])
            # PE consumes the identity in bf16 (0.0/1.0 are exact).
            ident = const_pool.tile([P, P], BF16)
            nc.scalar.copy(ident[:], ident_f[:])

            # Bias first on the gpsimd queue: the DVE bias-adds need b_rep
            # early, and the broadcast is quick; the w staging follows.
            # b_rep[p, h, j] = b[h, j].
            b_lin = const_pool.tile([1, NB, BW], FP32)
            nc.gpsimd.dma_start(
                b_lin[:], b.rearrange("(one h) j -> one h j", one=1)
            )
            b_rep = const_pool.tile([P, NB, BW], FP32)
            nc.gpsimd.partition_broadcast(
                b_rep[:].rearrange("p h j -> p (h j)"),
                b_lin[:].rearrange("o h j -> o (h j)"),
            )

            # w_sb[p, h, ic, j] = w[h, ic*128+p, j]; contiguous per partition
            # (host pre-transposed). Two chunks so the first matmuls only
            # wait for blocks 0-7.
            w_sb = const_pool.tile([P, NB, 2, BW], BF16)
            w_re = w.rearrange("p (h ic j) -> p h ic j", h=NB, ic=2)
            for q in range(2):
                nc.scalar.dma_start(
                    w_sb[:, 8 * q:8 * q + 8], w_re[:, 8 * q:8 * q + 8]
                )

            x_rows = x.rearrange("(t p) i -> t p i", p=P)
            y_rows = y.rearrange("(t p) i -> t p i", p=P)

            import contextlib

            rep_ctx = (
                tc.For_i(0, repeat, 1) if repeat > 1 else contextlib.nullcontext()
            )
            with rep_ctx:
                _main_loop(nc, tc, locals())

    nc.compile()
    return nc


def _main_loop(nc, tc, env):
    x_pool = env["x_pool"]
    xt_pool = env["xt_pool"]
    y_pool = env["y_pool"]
    psum_t = env["psum_t"]
    psum_y = env["psum_y"]
    ident = env["ident"]
    w_sb = env["w_sb"]
    b_rep = env["b_rep"]
    x_rows = env["x_rows"]
    y_rows = env["y_rows"]
    if True:
            for mi in range(M_TILES):
                # One contiguous 1 MB read: partition p <- row p (8 KB each).
                # Tile 0 loads in halves so the PE can start sooner.
                x_t = x_pool.tile([P, WIDTH], BF16)
                if mi == 0:
                    hw = WIDTH // 2
                    for q in range(2):
                        nc.sync.dma_start(
                            x_t[:, q * hw:(q + 1) * hw],
                            x_rows[mi][:, q * hw:(q + 1) * hw],
                        )
                else:
                    nc.sync.dma_start(x_t[:], x_rows[mi])

                # Per group g: transpose chunks 4g..4g+3 (features onto
                # partitions), then immediately the matmuls for blocks
                # 2g, 2g+1 which consume exactly those chunks.
                xT = xt_pool.tile([P, WIDTH // P, P], BF16)
                y_t = y_pool.tile([P, NB, BW], BF16)
                for g in range(8):
                    pt = psum_t.tile([P, 4, P], BF16, tag="pt")
                    for k in range(4):
                        c = 4 * g + k
                        nc.tensor.transpose(
                            pt[:, k, :], x_t[:, c * P:(c + 1) * P], ident[:]
                        )
                    nc.scalar.copy(xT[:, 4 * g:4 * g + 4, :], pt[:])

                    py = psum_y.tile([P, 2, BW], FP32)
                    for u in range(2):
                        h = 2 * g + u
                        nc.tensor.matmul(
                            py[:, u, :], xT[:, 2 * h, :], w_sb[:, h, 0, :],
                            start=True, stop=False,
                        )
                        nc.tensor.matmul(
                            py[:, u, :], xT[:, 2 * h + 1, :], w_sb[:, h, 1, :],
                            start=False, stop=True,
                        )
                    nc.vector.tensor_add(
                        y_t[:, 2 * g:2 * g + 2, :],
                        py[:],
                        b_rep[:, 2 * g:2 * g + 2, :],
                    )
                    if mi == M_TILES - 1:
                        # Last tile: stream each group's slice out right after
                        # its bias-add so the kernel tail is just one small
                        # DMA instead of a full row. Pool (gpsimd) ring: its
                        # sequencer is idle in steady state, so y issues never
                        # contend with the ACT evictions.
                        nc.gpsimd.dma_start(
                            y_rows[mi][:, g * 512:(g + 1) * 512],
                            y_t[:].rearrange("p h j -> p (h j)")[
                                :, g * 512:(g + 1) * 512
                            ],
                        )
                if mi != M_TILES - 1:
                    y_flat = y_t[:].rearrange("p h j -> p (h j)")
                    hw = WIDTH // 2
                    for q in range(2):
                        nc.gpsimd.dma_start(
                            y_rows[mi][:, q * hw:(q + 1) * hw],
                            y_flat[:, q * hw:(q + 1) * hw],
                        )


class _Runner:
    """Compile once, keep the jitted SPMD executable for reuse."""

    def __init__(self, repeat=1):
        import jax
        from jax.experimental.shard_map import shard_map
        from jax.sharding import Mesh, PartitionSpec

        self.jax = jax
        nc = _build(repeat=repeat)
        bass2jax.install_neuronx_cc_hook()

        assert nc.dbg_addr is None
        part_name = (
            nc.partition_id_tensor.name if nc.partition_id_tensor else None
        )
        in_names, out_names, out_avals = [], [], []
        for alloc in nc.m.functions[0].allocations:
            if not isinstance(alloc, mybir.MemoryLocationSet):
                continue
            name = alloc.memorylocations[0].name
            if alloc.kind == "ExternalInput":
                if name != part_name:
                    in_names.append(name)
            elif alloc.kind == "ExternalOutput":
                out_names.append(name)
                out_avals.append(
                    jax.core.ShapedArray(
                        tuple(alloc.tensor_shape), mybir.dt.np(alloc.dtype)
                    )
                )
        self.in_names = list(in_names)
        self.out_names = out_names
        self.out_avals = out_avals
        n_params = len(in_names)
        n_outs = len(out_names)
        all_names = in_names + out_names
        if part_name is not None:
            all_names = all_names + [part_name]

        def _body(*args):
            operands = list(args)
            if part_name is not None:
                operands.append(bass2jax.partition_id_tensor())
            outs = bass2jax._bass_exec_p.bind(
                *operands,
                out_avals=tuple(out_avals),
                in_names=tuple(all_names),
                out_names=tuple(out_names),
                lowering_input_output_aliases=(),
                sim_require_finite=True,
                sim_require_nnan=True,
                nc=nc,
            )
            return tuple(outs)

        devices = jax.devices()[:N_CORES]
        assert len(devices) == N_CORES
        self.mesh = Mesh(np.asarray(devices), ("core",))
        in_specs = (PartitionSpec("core"),) * (n_params + n_outs)
        out_specs = (PartitionSpec("core"),) * n_outs
        self.donate = tuple(range(n_params, n_params + n_outs))
        self.fn = jax.jit(
            shard_map(
                _body,
                mesh=self.mesh,
                in_specs=in_specs,
                out_specs=out_specs,
                check_rep=False,
            ),
            donate_argnums=self.donate,
            keep_unused=True,
        )

    def zeros(self):
        return [
            np.zeros((N_CORES * a.shape[0], *a.shape[1:]), a.dtype)
            for a in self.out_avals
        ]

    def prep(self, x, w, b):
        """Global (concatenated-over-cores) input arrays, in in_names order."""
        import jax
        import jax.numpy as jnp

        with jax.default_device(jax.devices("cpu")[0]):
            x2 = np.asarray(
                jnp.asarray(np.asarray(x).reshape(ROWS_TOTAL, WIDTH)).astype(
                    jnp.bfloat16
                )
            )
            wb = np.asarray(jnp.asarray(np.asarray(w)).astype(jnp.bfloat16))
        # w[h, ic*128+p, j] -> [p, (h, ic, j)]: per-partition-contiguous.
        wt = np.ascontiguousarray(
            wb.reshape(NB, 2, P, BW).transpose(2, 0, 1, 3).reshape(P, NB * 2 * BW)
        )
        b = np.ascontiguousarray(np.asarray(b, dtype=np.float32))
        per = {
            "x": x2,
            "w": np.concatenate([wt] * N_CORES, axis=0),
            "b": np.concatenate([b] * N_CORES, axis=0),
        }
        return [per[n] for n in self.in_names]

    def __call__(self, ins, zeros):
        outs = self.fn(*ins, *zeros)
        return dict(zip(self.out_names, outs))


_RUNNER = None


def _get_runner():
    global _RUNNER
    if _RUNNER is None:
        _RUNNER = _Runner()
    return _RUNNER


def kernel(x, w, b):
    r = _get_runner()
    outs = r(r.prep(x, w, b), r.zeros())
    y = np.asarray(outs["y"]).astype(np.float32)
    return y.reshape(4, 4096, WIDTH)
